# revision 24
# baseline (speedup 1.0000x reference)
"""GQA (16 query heads, 4 KV groups) forward kernel for 8 Trainium2 NeuronCores.

Sharding: core = (batch b in 0..1) x (kv-group g in 0..3).  Each core owns one
batch element and one whole KV group (4 query heads), computing the output
slice out[b, :, g*512:(g+1)*512].

Per-core plan (all matmul inputs fp16, fp32 PSUM accumulation):
  - x^T arrives from DRAM packed by t-chunk so K/V projections start as soon
    as the first 2.1MB chunk lands (DMA order: wk, wv, x0, wq, x1..x3).
  - K^T produced directly ([d, t], stationary Wk); V produced in natural
    layout ([t, d]) via x-stationary matmuls - no PE transposes anywhere.
  - Attention in transposed-score layout, two heads per pass so ACT exp and
    DVE sum-adds run on paired [128, 2, 512] tiles (half the instruction
    overhead).  Causal mask via gpsimd affine_select; exp restricted to the
    unmasked column range on diagonal blocks.
  - Softmax denominators via ones-vector matmul (partition reduction on PE),
    reciprocal on DVE, broadcast via gpsimd partition_broadcast, one DVE
    multiply to normalize - output stays in [d, q] layout and the host
    transposes it back during assemble.
  - Q^T chunks and later K/V projection chunks are interleaved as "filler"
    PE work inside the attention kb-loops so the PE never waits on ACT.
"""

import sys

if "/opt/trn_rl_repo" not in sys.path:
    sys.path.insert(0, "/opt/trn_rl_repo")

import numpy as np

import concourse.bass as bass
import concourse.mybir as mybir
import concourse.tile as tile
from concourse import bacc
from concourse.bass_utils import run_bass_kernel_spmd
from concourse.masks import make_identity

B, T, C = 2, 2048, 2048
HEADS, GROUPS = 16, 4
HD = C // HEADS          # 128 head dim
H2G = HEADS // GROUPS    # 4 query heads per group
DG = H2G * HD            # 512 output cols per core
DKV = HD                 # 128 kv dim per group
NCT = C // 128           # 16 contraction tiles
NQC = T // 512           # 4 query chunks (= t chunks)
NKB = T // 128           # 16 key blocks
SCALE = HD ** -0.5

F32 = mybir.dt.float32
FP16 = mybir.dt.float16


def _body(tc, xb, wqt, wkt, wvt, out_d):
    nc = tc.nc
    act_exp = mybir.ActivationFunctionType.Exp
    is_ge = mybir.AluOpType.is_ge
    alu_mult = mybir.AluOpType.mult

    with (
        tc.tile_pool(name="const", bufs=1) as cpool,
        tc.tile_pool(name="data", bufs=1) as data,
        tc.tile_pool(name="qt_sb", bufs=2) as qtsb,
        tc.tile_pool(name="ex_sb", bufs=6) as expool,
        tc.tile_pool(name="sum_sb", bufs=4) as sump,
        tc.tile_pool(name="o_sb", bufs=2) as outp,
        tc.tile_pool(name="vt_sb", bufs=2) as vtsb,
        tc.tile_pool(name="rb_sb", bufs=4) as rbp,
        tc.tile_pool(name="pv_ps", bufs=1, space="PSUM") as pvp,
        tc.tile_pool(name="st_ps", bufs=2, space="PSUM") as stp,
        tc.tile_pool(name="mi_ps", bufs=2, space="PSUM") as mip,
    ):
        ones_m = cpool.tile([128, 128], FP16)
        nc.vector.memset(ones_m[:], 1.0)

        xT = data.tile([128, NQC, NCT, 512], FP16)  # [c%128, tchunk, ci, t]
        wq = data.tile([128, H2G, NCT, 128], FP16)  # Wq^T tiles [c%128, h, ci, d]
        wk = data.tile([128, NCT, DKV], FP16)
        wv = data.tile([128, NCT, DKV], FP16)
        kT = data.tile([128, NQC, 512], FP16)       # K^T: [d, tchunk, t]
        vn = data.tile([128, NKB, DKV], FP16)       # V natural: [t%128, kb, d]

        # ---- input DMAs ----
        # sync HWDGE: startup-critical stream; scalar HWDGE: only cheap early
        # dispatches (they occupy the ACT queue in program order); gpsimd
        # SWDGE: bulk x2/x3, gated on x0 completion so they don't steal HBM
        # bandwidth from the startup-critical chunks.
        wkr = wkt.rearrange("(ci p) d -> p ci d", p=128)
        wvr = wvt.rearrange("(ci p) d -> p ci d", p=128)
        wqr = [wqt[h].rearrange("(ci p) d -> p ci d", p=128)
               for h in range(H2G)]
        for ci in range(NCT):
            eng = nc.sync if ci % 2 == 0 else nc.scalar
            eng.dma_start(out=wk[:, ci, :], in_=wkr[:, ci, :])
            eng.dma_start(out=wv[:, ci, :], in_=wvr[:, ci, :])
            eng.dma_start(out=wq[:, 0, ci, :], in_=wqr[0][:, ci, :])
            eng.dma_start(out=wq[:, 1, ci, :], in_=wqr[1][:, ci, :])
            eng.dma_start(out=xT[:, 0, ci, :], in_=xb[0, ci])
        for h in (2, 3):
            nc.sync.dma_start(out=wq[:, h, :, :], in_=wqr[h])
        for tcx in range(1, NQC):
            for ci in range(NCT):
                nc.sync.dma_start(out=xT[:, tcx, ci, :], in_=xb[tcx, ci])

        # ---- projection chunk emitters (each ~1-4us of PE work) ----
        def k_chunk(tcx):
            ps = mip.tile([128, 512], F32, tag="mi", name=f"kp{tcx}")
            for ci in range(NCT):
                nc.tensor.matmul(
                    ps[:], wk[:, ci, :], xT[:, tcx, ci, :],
                    start=(ci == 0), stop=(ci == NCT - 1))
            nc.vector.tensor_copy(kT[:, tcx, :], ps[:])

        def v_chunk(tcx):
            # V^T projection for the whole t-chunk, then one SBUF->SBUF DMA
            # transpose (xbar) into natural [t, d] layout - no PE transposes.
            ps = mip.tile([128, 512], F32, tag="mi", name=f"vp{tcx}")
            for ci in range(NCT):
                nc.tensor.matmul(
                    ps[:], wv[:, ci, :], xT[:, tcx, ci, :],
                    start=(ci == 0), stop=(ci == NCT - 1))
            vt = vtsb.tile([128, 512], FP16, tag="vt", name=f"vt{tcx}")
            nc.vector.tensor_copy(vt[:], ps[:])
            deng = nc.scalar if tcx < 2 else nc.sync
            deng.dma_start_transpose(
                out=vn[:, tcx * 4:(tcx + 1) * 4, :], in_=vt[:])

        vt_tiles = {}
        qt_tiles = {}

        def q_chunk(qc, h):
            if qc not in qt_tiles:
                qt_tiles[qc] = qtsb.tile(
                    [128, H2G, 512], FP16, tag="qt", name=f"qt{qc}")
            qt = qt_tiles[qc]
            ps = mip.tile([128, 512], F32, tag="mi", name=f"qp{qc}_{h}")
            for ci in range(NCT):
                nc.tensor.matmul(
                    ps[:], wq[:, h, ci, :],
                    xT[:, qc, ci, :],
                    start=(ci == 0), stop=(ci == NCT - 1))
            nc.vector.tensor_copy(qt[:, h, :], ps[:])
            return qt

        # filler queue: (stage, deadline_global_iter, emit_fn) where the
        # global iter for stage s counts hp*nkb_s + kb across its two passes.
        # qt heads 0/1 are needed at pass(s,0) start, heads 2/3 only at
        # pass(s,1) start; K(s)/V(s) only at the diagonal blocks of pass(s,0).
        fillers = [(0, 4, lambda h=h: q_chunk(0, h)) for h in (2, 3)]
        for s in range(1, NQC):
            nkb_s = 4 * s + 4
            for h in (0, 1):
                fillers.append((s, 0, lambda s=s, h=h: q_chunk(s, h)))
            fillers.append((s, max(0, 4 * s - 2), lambda s=s: k_chunk(s)))
            fillers.append((s, max(0, 4 * s - 2), lambda s=s: v_chunk(s)))
            for h in (2, 3):
                fillers.append((s, nkb_s, lambda s=s, h=h: q_chunk(s, h)))
        fillers.reverse()  # pop() from the front

        def drain_fillers(stage, itr):
            while fillers and (fillers[-1][0], fillers[-1][1]) <= (stage, itr):
                fillers.pop()[2]()

        def pop_filler():
            if fillers:
                fillers.pop()[2]()

        # ---- fused startup: K/V/Q^T-h0/h1 ci-loops interleaved so the PE
        # consumes each x0 chunk (+ its weight chunks) as it lands; Q^T
        # accumulators borrow the idle st-pool banks.
        qt0 = qtsb.tile([128, H2G, 512], FP16, tag="qt", name="qt0")
        qt_tiles[0] = qt0
        ps_k = mip.tile([128, 512], F32, tag="mi", name="kp0")
        ps_v = mip.tile([128, 512], F32, tag="mi", name="vp0")
        ps_q = [stp.tile([128, 2, 512], F32, tag="st", name=f"qp0_{h}")
                for h in (0, 1)]
        for ci in range(NCT):
            st_ci, sp_ci = (ci == 0), (ci == NCT - 1)
            nc.tensor.matmul(ps_k[:], wk[:, ci, :], xT[:, 0, ci, :],
                             start=st_ci, stop=sp_ci)
            nc.tensor.matmul(ps_v[:], wv[:, ci, :], xT[:, 0, ci, :],
                             start=st_ci, stop=sp_ci)
            for h in (0, 1):
                nc.tensor.matmul(ps_q[h][:, 0, :], wq[:, h, ci, :],
                                 xT[:, 0, ci, :], start=st_ci, stop=sp_ci)
        nc.vector.tensor_copy(kT[:, 0, :], ps_k[:])
        vt = vtsb.tile([128, 512], FP16, tag="vt", name="vt0")
        nc.vector.tensor_copy(vt[:], ps_v[:])
        nc.scalar.dma_start_transpose(out=vn[:, 0:4, :], in_=vt[:])
        for h in (0, 1):
            nc.vector.tensor_copy(qt0[:, h, :], ps_q[h][:, 0, :])

        # ---- attention: two heads per pass ----
        for qc in range(NQC):
            drain_fillers(qc, 0)
            qt = qt_tiles[qc]
            nkb = 4 * qc + 4
            for hp in range(2):
                sums = sump.tile([128, 2, 512], FP16, tag="sums",
                                 name=f"sums{qc}_{hp}")
                pv = [
                    pvp.tile([128, 512], F32, tag=f"pv{hh}", name=f"pv{qc}_{hp}_{hh}")
                    for hh in range(2)
                ]
                for kb in range(nkb):
                    git = hp * nkb + kb
                    drain_fillers(qc, git)
                    diag = kb >= 4 * qc
                    # columns < base are fully masked out: skip them in the
                    # scores matmul, exp, sum and PV entirely.
                    base = (kb - 4 * qc) * 128 if diag else 0
                    st = stp.tile([128, 2, 512], F32, tag="st",
                                  name=f"st{qc}_{hp}_{kb}")
                    kblk = kT[:, kb // 4, (kb % 4) * 128:(kb % 4 + 1) * 128]
                    for hh in range(2):
                        nc.tensor.matmul(
                            st[:, hh, base:], kblk, qt[:, 2 * hp + hh, base:],
                            start=True, stop=True)
                    ex = expool.tile([128, 2, 512], FP16, tag="ex",
                                     name=f"ex{qc}_{hp}_{kb}")
                    nc.scalar.activation(
                        ex[:, :, base:], st[:, :, base:], act_exp, scale=SCALE)
                    if diag:
                        # [base, base+128) is the triangular boundary strip;
                        # columns >= base+128 are fully kept.
                        for hh in range(2):
                            nc.gpsimd.affine_select(
                                out=ex[:, hh, base:base + 128],
                                in_=ex[:, hh, base:base + 128],
                                compare_op=is_ge,
                                fill=0.0,
                                base=0,
                                pattern=[[1, 128]],
                                channel_multiplier=-1,
                            )
                    if kb == 0:
                        nc.vector.tensor_copy(sums[:], ex[:])
                    else:
                        nc.vector.tensor_add(
                            sums[:, :, base:], sums[:, :, base:], ex[:, :, base:])
                    for hh in range(2):
                        nc.tensor.matmul(
                            pv[hh][:, base:], vn[:, kb, :], ex[:, hh, base:],
                            start=(kb == 0), stop=(kb == nkb - 1))
                    if git % 3 == 0:
                        pop_filler()
                # ---- wrap-up: denominators + normalize + store ----
                o_sb = outp.tile([128, 2, 512], FP16, tag="o",
                                 name=f"o{qc}_{hp}")
                for hh in range(2):
                    # ones_m.T @ sums = softmax denominator replicated across
                    # all 128 partitions, in one matmul
                    den = mip.tile([128, 512], F32, tag="mi",
                                   name=f"den{qc}_{hp}_{hh}")
                    nc.tensor.matmul(
                        den[:], ones_m[:], sums[:, hh, :],
                        start=True, stop=True)
                    rb = rbp.tile([128, 512], F32, tag="rb",
                                  name=f"rb{qc}_{hp}_{hh}")
                    nc.vector.reciprocal_approx_fast(rb[:], den[:])
                    nc.vector.tensor_tensor(
                        o_sb[:, hh, :], pv[hh][:], rb[:], op=alu_mult)
                oeng = nc.scalar if qc < 2 else nc.sync
                for hh in range(2):
                    oeng.dma_start(
                        out=out_d[hp * 256 + hh * 128:hp * 256 + (hh + 1) * 128,
                                  qc * 512:(qc + 1) * 512],
                        in_=o_sb[:, hh, :])
        # emit any leftover fillers (shouldn't happen)
        while fillers:
            fillers.pop()[2]()


def build_nc():
    # Bacc (not raw Bass): its finalize passes split multi-sem waits
    # (move_matmul_waits_to_ldweights / generate_event_semaphores) to meet the
    # 1-wait-per-instruction hardware constraint walrus enforces.
    nc = bacc.Bacc("TRN2", target_bir_lowering=False)
    # xb is x[b] pre-transposed on the host and packed by t-chunk:
    # [tchunk, ci, c%128, t] fp16
    xb = nc.declare_dram_parameter("xb", [NQC, NCT, 128, 512], FP16, isOutput=False)
    wqt = nc.declare_dram_parameter("wqt", [H2G, C, 128], FP16, isOutput=False)
    wkt = nc.declare_dram_parameter("wkt", [C, DKV], FP16, isOutput=False)
    wvt = nc.declare_dram_parameter("wvt", [C, DKV], FP16, isOutput=False)
    # out is stored [d, t] fp16; the host transposes/upcasts during assemble
    out_d = nc.declare_dram_parameter("out", [DG, T], FP16, isOutput=True)
    with tile.TileContext(nc) as tc:
        _body(tc, xb, wqt, wkt, wvt, out_d)
    nc.compile()
    return nc


def make_in_maps(x, Wq, Wk, Wv):
    f16 = np.float16
    in_maps = []
    for b in range(B):
        xT = np.ascontiguousarray(x[b].T).astype(f16)        # [C, T]
        xb4 = np.ascontiguousarray(
            xT.reshape(NCT, 128, NQC, 512).transpose(2, 0, 1, 3))
        for g in range(GROUPS):
            in_maps.append({
                "xb": xb4,
                "wqt": np.ascontiguousarray(
                    Wq[g * DG:(g + 1) * DG].T.reshape(C, H2G, 128)
                    .transpose(1, 0, 2)).astype(f16),
                "wkt": np.ascontiguousarray(Wk[g * DKV:(g + 1) * DKV].T).astype(f16),
                "wvt": np.ascontiguousarray(Wv[g * DKV:(g + 1) * DKV].T).astype(f16),
            })
    return in_maps


def assemble(results):
    out = np.empty((B, T, C), np.float32)
    for i, res in enumerate(results):
        b, g = divmod(i, GROUPS)
        out[b, :, g * DG:(g + 1) * DG] = res["out"].T.astype(np.float32)
    return out


def run(x, Wq, Wk, Wv, **spmd_kwargs):
    nc = build_nc()
    in_maps = make_in_maps(x, Wq, Wk, Wv)
    return run_bass_kernel_spmd(nc, in_maps, list(range(8)), **spmd_kwargs)


def kernel(x, Wq, Wk, Wv):
    return assemble(run(x, Wq, Wk, Wv).results)
